# revision 28
# baseline (speedup 1.0000x reference)
"""Additive (Bahdanau) alignment kernel for Trainium2, SPMD across 8 NeuronCores.

Model (per batch row b):
    dec_p = decoder_output @ W_dec.T + b_dec                  # (A,)
    enc_p = encoder_output[b] @ W_enc.T + b_enc               # (S, A)
    h     = tanh(dec_p + enc_p)                               # (S, A)
    scores= h @ V.T + b_v                                     # (S,)
    attn  = softmax(scores)                                   # (S,)
    ctx   = attn @ encoder_output[b]                          # (H,)
    out   = concat(ctx, decoder_output[b])                    # (2H,)

Strategy: data-parallel over batch (8 rows per core).  All large operands are
staged on the HOST into the exact fp8 partition-major layouts the matmuls
want -- enc twice ([h%128, hc, s] for the projection, [s%128, sub, h] for the
context reduction), W_enc.T / W_dec.T / dec.T / V as columns -- so the device
does nothing but big contiguous row-major DMA loads (8KB per partition per
batch row, full DMA bandwidth, ~16MB/core) and compute.  No XBAR transposes,
no on-device dtype casts.

Heavy matmuls run fp8e4m3 DoubleRow (W_enc and V are scaled x32 on the host
against e4m3's denormal cutoff; compensated in the activation scale).  The
Activation engine is the roofline here (tanh over S*A per row at 1 elem/lane/
cycle); the kernel keeps it saturated by pipelining pairs of 512-seq tiles:
PE projects pair g while ACT tanh's it, the V-dot of the PREVIOUS tiles runs
as 128-wide column matmuls (stationary = hT, moving = V) so the scores land
[s%128, block] across partitions and one exp per batch ROW handles all 2048
positions in a [128, 16] activation (~0.2us instead of 16 x 1us row-exps).
exp writes the e-columns straight to fp8 SBUF where they are the stationary
for the context matmuls; softmax needs no max pass (scores are ~|1.5|):
ctx_unnorm = sum exp(s - SHIFT) * enc and l = sum exp(s - SHIFT) (via the
activation accumulator + one ones-column matmul), normalized once per row.
"""

import numpy as np
from contextlib import ExitStack

import concourse.bass as bass
import concourse.mybir as mybir
import concourse.tile as tile
from concourse.vector_clock import ScopedClock
from concourse.bass import ts
from concourse.bass_utils import run_bass_kernel_spmd

F32 = mybir.dt.float32
BF16 = mybir.dt.bfloat16
FP8 = mybir.dt.float8e4
AF = mybir.ActivationFunctionType
DR = mybir.MatmulPerfMode.DoubleRow

N_CORES = 8
B, S, H, A = 64, 2048, 512, 512
B_SH = B // N_CORES
HC = H // 128            # h contraction chunks
AC = A // 128            # a chunks
ST = 512                 # seq tile (one PSUM bank at f32)
NP = S // (2 * ST)       # tile-pairs per batch row
SUB = ST // 128          # 128-row subtiles per seq tile
NSUB = S // 128          # subtiles per row
TOTAL = B_SH * NP

W_SCALE = 32.0  # W_enc / V values (~+-0.044) sit near e4m3's denormal range;
                # scale up before the fp8 cast, compensate in activation scale
SHIFT = 2.0     # exp(score - SHIFT) keeps e well inside fp8's normal range
N_WARM = 40     # junk matmuls at t=0 to open the PE HAM clock gate


class _SplitDrainTileContext(tile.TileContext):
    """This walrus build rejects instructions carrying more than a couple of
    semaphore waits ("Too many sync wait commands").  The stock TileContext
    tail puts every outstanding proc's wait on one Drain; split them across
    single-wait NOPs instead."""

    def _drain_and_barrier(self, tick_clock, wait_clock):
        nc = self.nc
        drain_inst = nc.sync.drain()
        wait_clock.add_sem_waits(
            drain_inst.ins, ScopedClock({None: tick_clock.global_clock})
        )
        si = drain_inst.ins.sync_info
        waits = list(si.on_wait)
        if len(waits) > 1:
            drain_inst.ins.sync_info = mybir.SyncInfo(
                on_wait=[waits[0]], on_update=list(si.on_update)
            )
            for w in waits[1:]:
                nop = nc.sync.nop(nofuse=True)
                nop.ins.sync_info = mybir.SyncInfo(on_wait=[w], on_update=[])

        nc.all_engine_barrier()
        assert self.sems is not None
        popped = nc._tile_sem_poison_stack.pop()
        assert popped is self._sem_poison
        # Distributed clear_and_free: the stock path emits one ~115ns
        # sem-clear per semaphore on a single queue (~6us serial tail for
        # ~50 sems).  Spread the clears round-robin over all five engines
        # (we are between two all-engine barriers, so this is race-free).
        sems = list(self.sems.allocated().values())
        if sems:
            sem_nums = [s.num if hasattr(s, "num") else s for s in sems]
            for r in bass.compact_to_ranges(sem_nums):
                assert nc._state.free_isdisjoint(r)
                nc.gpsimd.dma_reset(r)
            engines = [nc.gpsimd, nc.tensor, nc.vector, nc.scalar, nc.sync]
            for i, s in enumerate(sorted(sem_nums)):
                engines[i % len(engines)].sem_clear(range(s, s + 1))
            nc._state.prepend_free_semaphores(sem_nums)
            for poison_set in nc._tile_sem_poison_stack:
                poison_set.update(sem_nums)
        nc.all_engine_barrier()


def _split_excess_waits(nc, max_waits=1):
    """walrus (this build) rejects instructions with more than a couple of
    semaphore waits.  Move excess waits onto single-wait NOPs inserted just
    before the offending instruction on the same engine."""
    for fn in nc.m.functions:
        for bb in fn.blocks:
            new_insts = []
            for inst in bb.instructions:
                si = inst.sync_info
                waits = list(si.on_wait) if si is not None else []
                if len(waits) > max_waits:
                    head, keep = waits[:-max_waits], waits[-max_waits:]
                    for i, w in enumerate(head):
                        nop = mybir.InstNoOp(
                            name=f"{inst.name}-sw{i}",
                            engine=inst.engine,
                            bass_nofuse=True,
                            sync_info=mybir.SyncInfo(on_wait=[w], on_update=[]),
                        )
                        nc.register_instruction(nop, overwrite=True)
                        new_insts.append(nop)
                    inst.sync_info = mybir.SyncInfo(
                        on_wait=keep, on_update=list(si.on_update)
                    )
                new_insts.append(inst)
            bb.instructions[:] = new_insts


def build_nc():
    """Build the per-core Bass graph (SPMD: same graph on all cores)."""
    nc = bass.Bass("TRN2", target_bir_lowering=False, debug=False)
    dec = nc.declare_dram_parameter("decoder_output", (B_SH, H), F32, isOutput=False)
    etd = nc.declare_dram_parameter("enc_et", (B_SH, 128, NSUB, H), FP8, isOutput=False)
    encTd = nc.declare_dram_parameter("enc_tp", (B_SH, 128, HC, S), FP8, isOutput=False)
    WeTd = nc.declare_dram_parameter("WeT8", (128, HC, A), FP8, isOutput=False)
    WdTd = nc.declare_dram_parameter("WdT", (128, HC, A), BF16, isOutput=False)
    decTd = nc.declare_dram_parameter("decT", (128, HC, B_SH), BF16, isOutput=False)
    Vcd = nc.declare_dram_parameter("Vc8", (128, AC, 16), FP8, isOutput=False)
    bdd = nc.declare_dram_parameter("b_dec", (A,), F32, isOutput=False)
    bed = nc.declare_dram_parameter("b_enc", (A,), F32, isOutput=False)
    bvd = nc.declare_dram_parameter("bv_col", (128, 1), F32, isOutput=False)
    out = nc.declare_dram_parameter("out", (B_SH, 2 * H), F32, isOutput=True)

    with ExitStack() as ctx:
        tc = ctx.enter_context(_SplitDrainTileContext(nc))
        consts = ctx.enter_context(tc.tile_pool(name="consts", bufs=1))

        WeT8 = consts.tile([128, HC, A], FP8)
        Vc8 = consts.tile([128, AC, 16], FP8)
        WdT = consts.tile([128, HC, A], BF16)
        decT = consts.tile([128, HC, B_SH], BF16)
        dterm = consts.tile([128, AC, B_SH], F32)   # dec_p + b_dec + b_enc
        bv_raw = consts.tile([128, 1], F32)
        bvt = consts.tile([128, 1], F32)            # b_v - SHIFT, per partition
        bd_row = consts.tile([1, A], F32)
        be_row = consts.tile([1, A], F32)
        ecol8 = consts.tile([128, NP, 2 * SUB, 16], FP8)  # e columns, stride-16
        lacc = consts.tile([128, B_SH], F32)        # per-partition l partials
        ones_col = consts.tile([128, 1], F32)
        ones_row = consts.tile([1, B_SH], F32)
        junk = consts.tile([128, 128], BF16)
        zz = consts.tile([1, 2], F32)
        encT_sb = consts.tile([128, B_SH, HC, S], FP8)
        et_sb = consts.tile([128, B_SH, NSUB, H], FP8)

        # ---- small loads lead the sync ring so they land before the bulk
        # encoder streams start hogging the DMA engines (dterm and the first
        # projection gate on them) ----
        # dterm's inputs ride the ACT ring's HWDGE (they are needed ~8us in,
        # by the first tanh) so the sync ring reaches the proj-critical
        # encT row-0 load ~2.5us sooner.
        nc.scalar.dma_start(out=WdT, in_=WdTd[:, :, :])
        nc.scalar.dma_start(out=decT, in_=decTd[:, :, :])
        nc.scalar.dma_start(out=bd_row, in_=bdd[None, :])
        nc.scalar.dma_start(out=be_row, in_=bed[None, :])
        nc.sync.dma_start(out=WeT8, in_=WeTd[:, :, :])

        # ---- memsets / ACT table preload ----
        nc.gpsimd.memset(junk, 0.5)
        nc.gpsimd.memset(ones_col, 1.0)
        nc.gpsimd.memset(ones_row, 1.0)
        nc.gpsimd.memset(zz, 0.0)
        nc.scalar.activation(out=zz[:, 1:2], in_=zz[:, 0:1], func=AF.Tanh,
                             bias=zz[:, 0:1])

        # ---- big encoder streams on the sync ring (8KB/partition each).
        # encT rows lead et rows by ~1.5 rows: the projection consumes
        # encT[b] a full row-period before ctx needs et[b]. ----
        # row 0's first half (pair 0) lands ~1.5us before the full row
        # would, so the first projection starts that much earlier; the two
        # non-critical small loads ride behind it.
        nc.sync.dma_start(out=encT_sb[:, 0, :, 0:S//2], in_=encTd[0][:, :, 0:S//2])
        nc.sync.dma_start(out=encT_sb[:, 0, :, S//2:S], in_=encTd[0][:, :, S//2:S])
        nc.sync.dma_start(out=Vc8, in_=Vcd[:, :, :])
        nc.sync.dma_start(out=bv_raw, in_=bvd[:, :])
        nc.sync.dma_start(out=encT_sb[:, 1], in_=encTd[1])
        for b in range(B_SH):
            nc.sync.dma_start(out=et_sb[:, b], in_=etd[b])
            if b + 2 < B_SH:
                nc.sync.dma_start(out=encT_sb[:, b + 2], in_=encTd[b + 2])

        # decoder passthrough (DRAM -> DRAM)
        nc.gpsimd.dma_start(out=out[:, H : 2 * H], in_=dec[:, :])

        # ---- PE warm-up + dterm (scoped PSUM) ----
        with tc.tile_pool(name="wps", bufs=2, space="PSUM") as wps:
            warm = wps.tile([128, 128], F32, tag="warm")
            for _ in range(N_WARM):
                nc.tensor.matmul(warm, junk, junk, start=True, stop=True)
            for ac in range(AC):
                dt_ps = wps.tile([128, B_SH], F32, tag="dt")
                for hc in range(HC):
                    nc.tensor.matmul(
                        dt_ps, WdT[:, hc, ts(ac, 128)], decT[:, hc, :],
                        start=(hc == 0), stop=False,
                    )
                nc.tensor.matmul(dt_ps, bd_row[0:1, ts(ac, 128)], ones_row,
                                 start=False, stop=False)
                nc.tensor.matmul(dt_ps, be_row[0:1, ts(ac, 128)], ones_row,
                                 start=False, stop=True)
                nc.vector.tensor_copy(dterm[:, ac, :], dt_ps)
        nc.vector.tensor_scalar_add(bvt, bv_raw, -SHIFT)

        # ---- main pools ----
        hT_pool = ctx.enter_context(tc.tile_pool(name="hT", bufs=2))
        sm_pool = ctx.enter_context(tc.tile_pool(name="small", bufs=2))
        pp_pool = ctx.enter_context(tc.tile_pool(name="pp", bufs=2, space="PSUM"))
        scT_pool = ctx.enter_context(tc.tile_pool(name="scT", bufs=2, space="PSUM"))
        lps_pool = ctx.enter_context(tc.tile_pool(name="lps", bufs=1, space="PSUM"))
        ctx_pool = ctx.enter_context(tc.tile_pool(name="ctxp", bufs=1, space="PSUM"))

        WeT8r = WeT8.rearrange("p hc (ac f) -> p hc ac f", f=128)

        hTs = {}       # pair g -> hT tile
        scTs = {}      # row b -> scoresT psum tile [128, 16]
        ctxs = {}      # row b -> ctx psum tile [1, H]

        def emit_scoresT(g):
            """V-dot of pair g as column matmuls: stationary = hT slices
            (a-contraction), moving = V column, out [s%128, block]."""
            b, p = divmod(g, NP)
            if p == 0:
                scT = scT_pool.tile([128, 2 * 2 * SUB], F32, tag="scT")
                scTs[b] = scT
            scT = scTs[b]
            hT = hTs.pop(g)
            for sb in range(2 * SUB):
                t, csb = divmod(sb, SUB)
                col = 2 * SUB * p + sb
                for acp in (0, 2):
                    nc.tensor.matmul(
                        scT[:, col : col + 1],
                        hT[:, acp : acp + 2, t, ts(csb, 128)],
                        Vc8[:, acp : acp + 2, 0:1],
                        start=(acp == 0), stop=(acp == 2),
                        perf_mode=DR, skip_group_check=True,
                    )

        def emit_exp(b):
            """One exp for the whole row: [128, 16] -> e columns (fp8) +
            per-partition l partials via the activation accumulator."""
            nc.scalar.activation(
                out=ecol8[:, :, :, 0],
                in_=scTs[b],
                func=AF.Exp,
                bias=bvt,
                scale=1.0 / W_SCALE,
                accum_out=lacc[:, b : b + 1],
            )

        def emit_ctx_and_out(b):
            """Context reduction for row b (8 DR matmuls) + normalization."""
            ctx_ps = ctx_pool.tile([1, H], F32, tag="ctx")
            for p in range(NP):
                for q in range(SUB):
                    nc.tensor.matmul(
                        ctx_ps,
                        ecol8[:, p, 2 * q : 2 * q + 2, 0:1],
                        et_sb[:, b, 8 * p + 2 * q : 8 * p + 2 * q + 2, :],
                        start=(p == 0 and q == 0),
                        stop=(p == NP - 1 and q == SUB - 1),
                        perf_mode=DR, skip_group_check=True,
                    )
            l_ps = lps_pool.tile([1, 1], F32, tag="lps")
            nc.tensor.matmul(l_ps, lacc[:, b : b + 1], ones_col,
                             start=True, stop=True, skip_group_check=True)
            linv = sm_pool.tile([1, 1], F32, tag="linv")
            nc.vector.reciprocal(linv, l_ps)
            orow = sm_pool.tile([1, H], F32, tag="orow")
            nc.vector.tensor_scalar_mul(orow, ctx_ps, linv)
            nc.gpsimd.dma_start(out=out[b : b + 1, 0:H], in_=orow)

        for g in range(TOTAL):
            b, p = divmod(g, NP)
            hT = hT_pool.tile([128, AC, 2, ST], FP8, tag="hT")
            hTs[g] = hT
            for ac in range(AC):
                pp = pp_pool.tile([128, 2, ST], F32, tag="pp")
                for t in range(2):
                    for hq in (0, 2):
                        nc.tensor.matmul(
                            pp[:, t, :],
                            WeT8r[:, hq : hq + 2, ac, :],
                            encT_sb[:, b, hq : hq + 2, ts(2 * p + t, ST)],
                            start=(hq == 0), stop=(hq == 2),
                            perf_mode=DR,
                        )
                nc.scalar.activation(
                    out=hT[:, ac],
                    in_=pp,
                    func=AF.Tanh,
                    bias=dterm[:, ac, b : b + 1],
                    scale=1.0 / W_SCALE,
                )
                # V-dot of the previous pair rides between this pair's
                # PE/ACT work; row exp + ctx once both pairs are in.  At
                # g==1 the flush waits until this pair's projection is
                # fully emitted so the PE keeps streaming while the ACT
                # pipeline fills (scoresT(0) needs all of tanh(0)).
                flush_at = 3 if g == 1 else 0
                if ac == flush_at and g > 0:
                    emit_scoresT(g - 1)
                    if p == 0:
                        emit_exp(b - 1)
                if ac == min(flush_at + 1, 3) and p == 0 and b > 0:
                    emit_ctx_and_out(b - 1)

        emit_scoresT(TOTAL - 1)
        emit_exp(B_SH - 1)
        emit_ctx_and_out(B_SH - 1)

    _split_excess_waits(nc)
    return nc


_CACHED = {}


def _get_nc():
    if "nc" not in _CACHED:
        _CACHED["nc"] = build_nc()
    return _CACHED["nc"]


def _prep(inputs):
    """Host-side staging: fp8 partition-major copies of the big operands in
    the layouts the device matmuls consume directly."""
    import ml_dtypes

    f8 = ml_dtypes.float8_e4m3
    bf = ml_dtypes.bfloat16

    dec = np.ascontiguousarray(np.asarray(inputs["decoder_output"], np.float32))
    enc = np.asarray(inputs["encoder_output"], np.float32)
    Wd = np.asarray(inputs["W_dec"], np.float32)
    We = np.asarray(inputs["W_enc"], np.float32)
    bd = np.ascontiguousarray(np.asarray(inputs["b_dec"], np.float32))
    be = np.ascontiguousarray(np.asarray(inputs["b_enc"], np.float32))
    V = np.asarray(inputs["V"], np.float32)
    bv = np.asarray(inputs["b_v"], np.float32)

    enc8 = enc.astype(f8)
    # [b, s%128... ] -> partition-major: et[b, p, sub, h] = enc[b, sub*128+p, h]
    et = np.ascontiguousarray(enc8.reshape(B, NSUB, 128, H).transpose(0, 2, 1, 3))
    # encT[b, p, hc, s] = enc[b, s, hc*128+p]
    encT = np.ascontiguousarray(
        enc8.transpose(0, 2, 1).reshape(B, HC, 128, S).transpose(0, 2, 1, 3)
    )
    WeT8 = np.ascontiguousarray(
        (We.T * W_SCALE).reshape(HC, 128, A).transpose(1, 0, 2)
    ).astype(f8)
    WdT = np.ascontiguousarray(
        Wd.T.reshape(HC, 128, A).transpose(1, 0, 2)
    ).astype(bf)
    Vc8 = np.zeros((128, AC, 16), f8)
    Vc8[:, :, 0] = ((V[0] * W_SCALE).reshape(AC, 128).T).astype(f8)
    bv_col = np.ascontiguousarray(
        np.broadcast_to(bv.reshape(1, 1), (128, 1)).astype(np.float32)
    )
    return {
        "dec": dec, "et": et, "encT": encT, "WeT8": WeT8, "WdT": WdT,
        "Vc8": Vc8, "bd": bd, "be": be, "bv_col": bv_col,
    }


def make_in_maps(ins):
    import ml_dtypes

    bf = ml_dtypes.bfloat16
    in_maps = []
    for c in range(N_CORES):
        sl = slice(c * B_SH, (c + 1) * B_SH)
        dec_c = ins["dec"][sl]
        decT_c = np.ascontiguousarray(
            dec_c.T.reshape(HC, 128, B_SH).transpose(1, 0, 2)
        ).astype(bf)
        in_maps.append(
            {
                "decoder_output": dec_c,
                "enc_et": ins["et"][sl],
                "enc_tp": ins["encT"][sl],
                "WeT8": ins["WeT8"],
                "WdT": ins["WdT"],
                "decT": decT_c,
                "Vc8": ins["Vc8"],
                "b_dec": ins["bd"],
                "b_enc": ins["be"],
                "bv_col": ins["bv_col"],
            }
        )
    return in_maps


def kernel(**inputs) -> np.ndarray:
    ins = _prep(inputs)
    nc = _get_nc()
    in_maps = make_in_maps(ins)
    # The device occasionally comes up wedged from a previous process
    # (NRT_EXEC_UNIT_UNRECOVERABLE); a failed attempt clears it, so retry.
    last_err = None
    for _attempt in range(3):
        try:
            res = run_bass_kernel_spmd(nc, in_maps, core_ids=list(range(N_CORES)))
            return np.concatenate(
                [res.results[c]["out"] for c in range(N_CORES)], axis=0
            )
        except Exception as e:  # noqa: BLE001
            last_err = e
            import time

            time.sleep(5)
    raise last_err


# revision 29
# speedup vs baseline: 1.0587x; 1.0587x over previous
"""Additive (Bahdanau) alignment kernel for Trainium2, SPMD across 8 NeuronCores.

Model (per batch row b):
    dec_p = decoder_output @ W_dec.T + b_dec                  # (A,)
    enc_p = encoder_output[b] @ W_enc.T + b_enc               # (S, A)
    h     = tanh(dec_p + enc_p)                               # (S, A)
    scores= h @ V.T + b_v                                     # (S,)
    attn  = softmax(scores)                                   # (S,)
    ctx   = attn @ encoder_output[b]                          # (H,)
    out   = concat(ctx, decoder_output[b])                    # (2H,)

Strategy: data-parallel over batch (8 rows per core).  All large operands are
staged on the HOST into the exact fp8 partition-major layouts the matmuls
want -- enc twice ([h%128, hc, s] for the projection, [s%128, sub, h] for the
context reduction), W_enc.T / W_dec.T / dec.T / V as columns -- so the device
does nothing but big contiguous row-major DMA loads (8KB per partition per
batch row, full DMA bandwidth, ~16MB/core) and compute.  No XBAR transposes,
no on-device dtype casts.

Heavy matmuls run fp8e4m3 DoubleRow (W_enc and V are scaled x32 on the host
against e4m3's denormal cutoff; compensated in the activation scale).  The
Activation engine is the roofline here (tanh over S*A per row at 1 elem/lane/
cycle); the kernel keeps it saturated by pipelining pairs of 512-seq tiles:
PE projects pair g while ACT tanh's it, the V-dot of the PREVIOUS tiles runs
as 128-wide column matmuls (stationary = hT, moving = V) so the scores land
[s%128, block] across partitions and one exp per batch ROW handles all 2048
positions in a [128, 16] activation (~0.2us instead of 16 x 1us row-exps).
exp writes the e-columns straight to fp8 SBUF where they are the stationary
for the context matmuls; softmax needs no max pass (scores are ~|1.5|):
ctx_unnorm = sum exp(s - SHIFT) * enc and l = sum exp(s - SHIFT) (via the
activation accumulator + one ones-column matmul), normalized once per row.
"""

import numpy as np
from contextlib import ExitStack

import concourse.bass as bass
import concourse.mybir as mybir
import concourse.tile as tile
from concourse.vector_clock import ScopedClock
from concourse.bass import ts
from concourse.bass_utils import run_bass_kernel_spmd

F32 = mybir.dt.float32
BF16 = mybir.dt.bfloat16
FP8 = mybir.dt.float8e4
AF = mybir.ActivationFunctionType
DR = mybir.MatmulPerfMode.DoubleRow

N_CORES = 8
B, S, H, A = 64, 2048, 512, 512
B_SH = B // N_CORES
HC = H // 128            # h contraction chunks
AC = A // 128            # a chunks
ST = 512                 # seq tile (one PSUM bank at f32)
NP = S // (2 * ST)       # tile-pairs per batch row
SUB = ST // 128          # 128-row subtiles per seq tile
NSUB = S // 128          # subtiles per row
TOTAL = B_SH * NP

W_SCALE = 32.0  # W_enc / V values (~+-0.044) sit near e4m3's denormal range;
                # scale up before the fp8 cast, compensate in activation scale
SHIFT = 2.0     # exp(score - SHIFT) keeps e well inside fp8's normal range
N_WARM = 40     # junk matmuls at t=0 to open the PE HAM clock gate


class _SplitDrainTileContext(tile.TileContext):
    """This walrus build rejects instructions carrying more than a couple of
    semaphore waits ("Too many sync wait commands").  The stock TileContext
    tail puts every outstanding proc's wait on one Drain; split them across
    single-wait NOPs instead."""

    def _drain_and_barrier(self, tick_clock, wait_clock):
        nc = self.nc
        drain_inst = nc.sync.drain()
        wait_clock.add_sem_waits(
            drain_inst.ins, ScopedClock({None: tick_clock.global_clock})
        )
        si = drain_inst.ins.sync_info
        waits = list(si.on_wait)
        if len(waits) > 1:
            drain_inst.ins.sync_info = mybir.SyncInfo(
                on_wait=[waits[0]], on_update=list(si.on_update)
            )
            for w in waits[1:]:
                nop = nc.sync.nop(nofuse=True)
                nop.ins.sync_info = mybir.SyncInfo(on_wait=[w], on_update=[])

        nc.all_engine_barrier()
        assert self.sems is not None
        popped = nc._tile_sem_poison_stack.pop()
        assert popped is self._sem_poison
        # Distributed clear_and_free: the stock path emits one ~115ns
        # sem-clear per semaphore on a single queue (~6us serial tail for
        # ~50 sems).  Spread the clears round-robin over all five engines
        # (we are between two all-engine barriers, so this is race-free).
        sems = list(self.sems.allocated().values())
        if sems:
            sem_nums = [s.num if hasattr(s, "num") else s for s in sems]
            for r in bass.compact_to_ranges(sem_nums):
                assert nc._state.free_isdisjoint(r)
                nc.gpsimd.dma_reset(r)
            engines = [nc.gpsimd, nc.tensor, nc.vector, nc.scalar, nc.sync]
            for i, s in enumerate(sorted(sem_nums)):
                engines[i % len(engines)].sem_clear(range(s, s + 1))
            nc._state.prepend_free_semaphores(sem_nums)
            for poison_set in nc._tile_sem_poison_stack:
                poison_set.update(sem_nums)
        nc.all_engine_barrier()


def _split_excess_waits(nc, max_waits=1):
    """walrus (this build) rejects instructions with more than a couple of
    semaphore waits.  Move excess waits onto single-wait NOPs inserted just
    before the offending instruction on the same engine."""
    for fn in nc.m.functions:
        for bb in fn.blocks:
            new_insts = []
            for inst in bb.instructions:
                si = inst.sync_info
                waits = list(si.on_wait) if si is not None else []
                if len(waits) > max_waits:
                    head, keep = waits[:-max_waits], waits[-max_waits:]
                    for i, w in enumerate(head):
                        nop = mybir.InstNoOp(
                            name=f"{inst.name}-sw{i}",
                            engine=inst.engine,
                            bass_nofuse=True,
                            sync_info=mybir.SyncInfo(on_wait=[w], on_update=[]),
                        )
                        nc.register_instruction(nop, overwrite=True)
                        new_insts.append(nop)
                    inst.sync_info = mybir.SyncInfo(
                        on_wait=keep, on_update=list(si.on_update)
                    )
                new_insts.append(inst)
            bb.instructions[:] = new_insts


def build_nc():
    """Build the per-core Bass graph (SPMD: same graph on all cores)."""
    nc = bass.Bass("TRN2", target_bir_lowering=False, debug=False)
    dec = nc.declare_dram_parameter("decoder_output", (B_SH, H), F32, isOutput=False)
    etd = nc.declare_dram_parameter("enc_et", (B_SH, 128, NSUB, H), FP8, isOutput=False)
    encTd = nc.declare_dram_parameter("enc_tp", (B_SH, 128, HC, S), FP8, isOutput=False)
    WeTd = nc.declare_dram_parameter("WeT8", (128, HC, A), FP8, isOutput=False)
    WdTd = nc.declare_dram_parameter("WdT", (128, HC, A), BF16, isOutput=False)
    decTd = nc.declare_dram_parameter("decT", (128, HC, B_SH), BF16, isOutput=False)
    Vcd = nc.declare_dram_parameter("Vc8", (128, AC, 16), FP8, isOutput=False)
    bdd = nc.declare_dram_parameter("b_dec", (A,), F32, isOutput=False)
    bed = nc.declare_dram_parameter("b_enc", (A,), F32, isOutput=False)
    bvd = nc.declare_dram_parameter("bv_col", (128, 1), F32, isOutput=False)
    out = nc.declare_dram_parameter("out", (B_SH, 2 * H), F32, isOutput=True)

    with ExitStack() as ctx:
        tc = ctx.enter_context(_SplitDrainTileContext(nc))
        consts = ctx.enter_context(tc.tile_pool(name="consts", bufs=1))

        WeT8 = consts.tile([128, HC, A], FP8)
        Vc8 = consts.tile([128, AC, 16], FP8)
        WdT = consts.tile([128, HC, A], BF16)
        decT = consts.tile([128, HC, B_SH], BF16)
        dterm = consts.tile([128, AC, B_SH], F32)   # dec_p + b_dec + b_enc
        bv_raw = consts.tile([128, 1], F32)
        bvt = consts.tile([128, 1], F32)            # b_v - SHIFT, per partition
        bd_row = consts.tile([1, A], F32)
        be_row = consts.tile([1, A], F32)
        ecol8 = consts.tile([128, NP, 2 * SUB, 16], FP8)  # e columns, stride-16
        lacc = consts.tile([128, B_SH], F32)        # per-partition l partials
        ones_col = consts.tile([128, 1], F32)
        ones_row = consts.tile([1, B_SH], F32)
        junk = consts.tile([128, 128], BF16)
        zz = consts.tile([1, 2], F32)
        encT_sb = consts.tile([128, B_SH, HC, S], FP8)
        et_sb = consts.tile([128, B_SH, NSUB, H], FP8)

        # ---- small loads lead the sync ring so they land before the bulk
        # encoder streams start hogging the DMA engines (dterm and the first
        # projection gate on them) ----
        nc.sync.dma_start(out=WeT8, in_=WeTd[:, :, :])
        nc.sync.dma_start(out=WdT, in_=WdTd[:, :, :])
        nc.sync.dma_start(out=decT, in_=decTd[:, :, :])
        nc.sync.dma_start(out=bd_row, in_=bdd[None, :])
        nc.sync.dma_start(out=be_row, in_=bed[None, :])

        # ---- memsets / ACT table preload ----
        nc.gpsimd.memset(junk, 0.5)
        nc.gpsimd.memset(ones_col, 1.0)
        nc.gpsimd.memset(ones_row, 1.0)
        nc.gpsimd.memset(zz, 0.0)
        nc.scalar.activation(out=zz[:, 1:2], in_=zz[:, 0:1], func=AF.Tanh,
                             bias=zz[:, 0:1])

        # ---- big encoder streams on the sync ring (8KB/partition each).
        # encT rows lead et rows by ~1.5 rows: the projection consumes
        # encT[b] a full row-period before ctx needs et[b]. ----
        # row 0's first half (pair 0) lands ~1.5us before the full row
        # would, so the first projection starts that much earlier; the two
        # non-critical small loads ride behind it.
        nc.sync.dma_start(out=encT_sb[:, 0, :, 0:S//2], in_=encTd[0][:, :, 0:S//2])
        nc.sync.dma_start(out=encT_sb[:, 0, :, S//2:S], in_=encTd[0][:, :, S//2:S])
        nc.sync.dma_start(out=Vc8, in_=Vcd[:, :, :])
        nc.sync.dma_start(out=bv_raw, in_=bvd[:, :])
        nc.sync.dma_start(out=encT_sb[:, 1], in_=encTd[1])
        for b in range(B_SH):
            nc.sync.dma_start(out=et_sb[:, b], in_=etd[b])
            if b + 2 < B_SH:
                nc.sync.dma_start(out=encT_sb[:, b + 2], in_=encTd[b + 2])

        # decoder passthrough (DRAM -> DRAM)
        nc.gpsimd.dma_start(out=out[:, H : 2 * H], in_=dec[:, :])

        # ---- PE warm-up + dterm (scoped PSUM) ----
        with tc.tile_pool(name="wps", bufs=2, space="PSUM") as wps:
            warm = wps.tile([128, 128], F32, tag="warm")
            for _ in range(N_WARM):
                nc.tensor.matmul(warm, junk, junk, start=True, stop=True)
            for ac in range(AC):
                dt_ps = wps.tile([128, B_SH], F32, tag="dt")
                for hc in range(HC):
                    nc.tensor.matmul(
                        dt_ps, WdT[:, hc, ts(ac, 128)], decT[:, hc, :],
                        start=(hc == 0), stop=False,
                    )
                nc.tensor.matmul(dt_ps, bd_row[0:1, ts(ac, 128)], ones_row,
                                 start=False, stop=False)
                nc.tensor.matmul(dt_ps, be_row[0:1, ts(ac, 128)], ones_row,
                                 start=False, stop=True)
                nc.vector.tensor_copy(dterm[:, ac, :], dt_ps)
        nc.vector.tensor_scalar_add(bvt, bv_raw, -SHIFT)

        # ---- main pools ----
        hT_pool = ctx.enter_context(tc.tile_pool(name="hT", bufs=2))
        sm_pool = ctx.enter_context(tc.tile_pool(name="small", bufs=2))
        pp_pool = ctx.enter_context(tc.tile_pool(name="pp", bufs=2, space="PSUM"))
        scT_pool = ctx.enter_context(tc.tile_pool(name="scT", bufs=2, space="PSUM"))
        lps_pool = ctx.enter_context(tc.tile_pool(name="lps", bufs=1, space="PSUM"))
        ctx_pool = ctx.enter_context(tc.tile_pool(name="ctxp", bufs=1, space="PSUM"))

        WeT8r = WeT8.rearrange("p hc (ac f) -> p hc ac f", f=128)

        hTs = {}       # pair g -> hT tile
        scTs = {}      # row b -> scoresT psum tile [128, 16]
        ctxs = {}      # row b -> ctx psum tile [1, H]

        def emit_scoresT(g):
            """V-dot of pair g as column matmuls: stationary = hT slices
            (a-contraction), moving = V column, out [s%128, block]."""
            b, p = divmod(g, NP)
            if p == 0:
                scT = scT_pool.tile([128, 2 * 2 * SUB], F32, tag="scT")
                scTs[b] = scT
            scT = scTs[b]
            hT = hTs.pop(g)
            for sb in range(2 * SUB):
                t, csb = divmod(sb, SUB)
                col = 2 * SUB * p + sb
                for acp in (0, 2):
                    nc.tensor.matmul(
                        scT[:, col : col + 1],
                        hT[:, acp : acp + 2, t, ts(csb, 128)],
                        Vc8[:, acp : acp + 2, 0:1],
                        start=(acp == 0), stop=(acp == 2),
                        perf_mode=DR, skip_group_check=True,
                    )

        def emit_exp(b):
            """One exp for the whole row: [128, 16] -> e columns (fp8) +
            per-partition l partials via the activation accumulator."""
            nc.scalar.activation(
                out=ecol8[:, :, :, 0],
                in_=scTs[b],
                func=AF.Exp,
                bias=bvt,
                scale=1.0 / W_SCALE,
                accum_out=lacc[:, b : b + 1],
            )

        def emit_ctx_and_out(b):
            """Context reduction for row b (8 DR matmuls) + normalization."""
            ctx_ps = ctx_pool.tile([1, H], F32, tag="ctx")
            for p in range(NP):
                for q in range(SUB):
                    nc.tensor.matmul(
                        ctx_ps,
                        ecol8[:, p, 2 * q : 2 * q + 2, 0:1],
                        et_sb[:, b, 8 * p + 2 * q : 8 * p + 2 * q + 2, :],
                        start=(p == 0 and q == 0),
                        stop=(p == NP - 1 and q == SUB - 1),
                        perf_mode=DR, skip_group_check=True,
                    )
            l_ps = lps_pool.tile([1, 1], F32, tag="lps")
            nc.tensor.matmul(l_ps, lacc[:, b : b + 1], ones_col,
                             start=True, stop=True, skip_group_check=True)
            linv = sm_pool.tile([1, 1], F32, tag="linv")
            nc.vector.reciprocal(linv, l_ps)
            orow = sm_pool.tile([1, H], F32, tag="orow")
            nc.vector.tensor_scalar_mul(orow, ctx_ps, linv)
            nc.gpsimd.dma_start(out=out[b : b + 1, 0:H], in_=orow)

        for g in range(TOTAL):
            b, p = divmod(g, NP)
            hT = hT_pool.tile([128, AC, 2, ST], FP8, tag="hT")
            hTs[g] = hT
            for ac in range(AC):
                pp = pp_pool.tile([128, 2, ST], F32, tag="pp")
                for t in range(2):
                    for hq in (0, 2):
                        nc.tensor.matmul(
                            pp[:, t, :],
                            WeT8r[:, hq : hq + 2, ac, :],
                            encT_sb[:, b, hq : hq + 2, ts(2 * p + t, ST)],
                            start=(hq == 0), stop=(hq == 2),
                            perf_mode=DR,
                        )
                nc.scalar.activation(
                    out=hT[:, ac],
                    in_=pp,
                    func=AF.Tanh,
                    bias=dterm[:, ac, b : b + 1],
                    scale=1.0 / W_SCALE,
                )
                # V-dot of the previous pair rides between this pair's
                # PE/ACT work; row exp + ctx once both pairs are in.  At
                # g==1 the flush waits until this pair's projection is
                # fully emitted so the PE keeps streaming while the ACT
                # pipeline fills (scoresT(0) needs all of tanh(0)).
                flush_at = 3 if g == 1 else 0
                if ac == flush_at and g > 0:
                    emit_scoresT(g - 1)
                    if p == 0:
                        emit_exp(b - 1)
                if ac == min(flush_at + 1, 3) and p == 0 and b > 0:
                    emit_ctx_and_out(b - 1)

        emit_scoresT(TOTAL - 1)
        emit_exp(B_SH - 1)
        emit_ctx_and_out(B_SH - 1)

    _split_excess_waits(nc)
    return nc


_CACHED = {}


def _get_nc():
    if "nc" not in _CACHED:
        _CACHED["nc"] = build_nc()
    return _CACHED["nc"]


def _prep(inputs):
    """Host-side staging: fp8 partition-major copies of the big operands in
    the layouts the device matmuls consume directly."""
    import ml_dtypes

    f8 = ml_dtypes.float8_e4m3
    bf = ml_dtypes.bfloat16

    dec = np.ascontiguousarray(np.asarray(inputs["decoder_output"], np.float32))
    enc = np.asarray(inputs["encoder_output"], np.float32)
    Wd = np.asarray(inputs["W_dec"], np.float32)
    We = np.asarray(inputs["W_enc"], np.float32)
    bd = np.ascontiguousarray(np.asarray(inputs["b_dec"], np.float32))
    be = np.ascontiguousarray(np.asarray(inputs["b_enc"], np.float32))
    V = np.asarray(inputs["V"], np.float32)
    bv = np.asarray(inputs["b_v"], np.float32)

    enc8 = enc.astype(f8)
    # [b, s%128... ] -> partition-major: et[b, p, sub, h] = enc[b, sub*128+p, h]
    et = np.ascontiguousarray(enc8.reshape(B, NSUB, 128, H).transpose(0, 2, 1, 3))
    # encT[b, p, hc, s] = enc[b, s, hc*128+p]
    encT = np.ascontiguousarray(
        enc8.transpose(0, 2, 1).reshape(B, HC, 128, S).transpose(0, 2, 1, 3)
    )
    WeT8 = np.ascontiguousarray(
        (We.T * W_SCALE).reshape(HC, 128, A).transpose(1, 0, 2)
    ).astype(f8)
    WdT = np.ascontiguousarray(
        Wd.T.reshape(HC, 128, A).transpose(1, 0, 2)
    ).astype(bf)
    Vc8 = np.zeros((128, AC, 16), f8)
    Vc8[:, :, 0] = ((V[0] * W_SCALE).reshape(AC, 128).T).astype(f8)
    bv_col = np.ascontiguousarray(
        np.broadcast_to(bv.reshape(1, 1), (128, 1)).astype(np.float32)
    )
    return {
        "dec": dec, "et": et, "encT": encT, "WeT8": WeT8, "WdT": WdT,
        "Vc8": Vc8, "bd": bd, "be": be, "bv_col": bv_col,
    }


def make_in_maps(ins):
    import ml_dtypes

    bf = ml_dtypes.bfloat16
    in_maps = []
    for c in range(N_CORES):
        sl = slice(c * B_SH, (c + 1) * B_SH)
        dec_c = ins["dec"][sl]
        decT_c = np.ascontiguousarray(
            dec_c.T.reshape(HC, 128, B_SH).transpose(1, 0, 2)
        ).astype(bf)
        in_maps.append(
            {
                "decoder_output": dec_c,
                "enc_et": ins["et"][sl],
                "enc_tp": ins["encT"][sl],
                "WeT8": ins["WeT8"],
                "WdT": ins["WdT"],
                "decT": decT_c,
                "Vc8": ins["Vc8"],
                "b_dec": ins["bd"],
                "b_enc": ins["be"],
                "bv_col": ins["bv_col"],
            }
        )
    return in_maps


def kernel(**inputs) -> np.ndarray:
    ins = _prep(inputs)
    nc = _get_nc()
    in_maps = make_in_maps(ins)
    # The device occasionally comes up wedged from a previous process
    # (NRT_EXEC_UNIT_UNRECOVERABLE); a failed attempt clears it, so retry.
    last_err = None
    for _attempt in range(3):
        try:
            res = run_bass_kernel_spmd(nc, in_maps, core_ids=list(range(N_CORES)))
            return np.concatenate(
                [res.results[c]["out"] for c in range(N_CORES)], axis=0
            )
        except Exception as e:  # noqa: BLE001
            last_err = e
            import time

            time.sleep(5)
    raise last_err


# revision 32
# speedup vs baseline: 1.1246x; 1.0623x over previous
"""Additive (Bahdanau) alignment kernel for Trainium2, SPMD across 8 NeuronCores.

Model (per batch row b):
    dec_p = decoder_output @ W_dec.T + b_dec                  # (A,)
    enc_p = encoder_output[b] @ W_enc.T + b_enc               # (S, A)
    h     = tanh(dec_p + enc_p)                               # (S, A)
    scores= h @ V.T + b_v                                     # (S,)
    attn  = softmax(scores)                                   # (S,)
    ctx   = attn @ encoder_output[b]                          # (H,)
    out   = concat(ctx, decoder_output[b])                    # (2H,)

Strategy: data-parallel over batch (8 rows per core).  All large operands are
staged on the HOST into the exact fp8 partition-major layouts the matmuls
want -- enc twice ([h%128, hc, s] for the projection, [s%128, sub, h] for the
context reduction), W_enc.T / W_dec.T / dec.T / V as columns -- so the device
does nothing but big contiguous row-major DMA loads (8KB per partition per
batch row, full DMA bandwidth, ~16MB/core) and compute.  No XBAR transposes,
no on-device dtype casts.

Heavy matmuls run fp8e4m3 DoubleRow (W_enc and V are scaled x32 on the host
against e4m3's denormal cutoff; compensated in the activation scale).  The
Activation engine is the roofline here (tanh over S*A per row at 1 elem/lane/
cycle); the kernel keeps it saturated by pipelining pairs of 512-seq tiles:
PE projects pair g while ACT tanh's it, the V-dot of the PREVIOUS tiles runs
as 128-wide column matmuls (stationary = hT, moving = V) so the scores land
[s%128, block] across partitions and one exp per batch ROW handles all 2048
positions in a [128, 16] activation (~0.2us instead of 16 x 1us row-exps).
exp writes the e-columns straight to fp8 SBUF where they are the stationary
for the context matmuls; softmax needs no max pass (scores are ~|1.5|):
ctx_unnorm = sum exp(s - SHIFT) * enc and l = sum exp(s - SHIFT) (via the
activation accumulator + one ones-column matmul), normalized once per row.
"""

import numpy as np
from contextlib import ExitStack

import concourse.bass as bass
import concourse.mybir as mybir
import concourse.tile as tile
from concourse.vector_clock import ScopedClock
from concourse.bass import ts
from concourse.bass_utils import run_bass_kernel_spmd

F32 = mybir.dt.float32
BF16 = mybir.dt.bfloat16
FP8 = mybir.dt.float8e4
AF = mybir.ActivationFunctionType
DR = mybir.MatmulPerfMode.DoubleRow

N_CORES = 8
B, S, H, A = 64, 2048, 512, 512
B_SH = B // N_CORES
HC = H // 128            # h contraction chunks
AC = A // 128            # a chunks
ST = 512                 # seq tile (one PSUM bank at f32)
NP = S // (2 * ST)       # tile-pairs per batch row
SUB = ST // 128          # 128-row subtiles per seq tile
NSUB = S // 128          # subtiles per row
TOTAL = B_SH * NP

W_SCALE = 32.0  # W_enc / V values (~+-0.044) sit near e4m3's denormal range;
                # scale up before the fp8 cast, compensate in activation scale
SHIFT = 2.0     # exp(score - SHIFT) keeps e well inside fp8's normal range
N_WARM = 40     # junk matmuls at t=0 to open the PE HAM clock gate


class _SplitDrainTileContext(tile.TileContext):
    """This walrus build rejects instructions carrying more than a couple of
    semaphore waits ("Too many sync wait commands").  The stock TileContext
    tail puts every outstanding proc's wait on one Drain; split them across
    single-wait NOPs instead."""

    def _drain_and_barrier(self, tick_clock, wait_clock):
        nc = self.nc
        drain_inst = nc.sync.drain()
        wait_clock.add_sem_waits(
            drain_inst.ins, ScopedClock({None: tick_clock.global_clock})
        )
        si = drain_inst.ins.sync_info
        waits = list(si.on_wait)
        if len(waits) > 1:
            drain_inst.ins.sync_info = mybir.SyncInfo(
                on_wait=[waits[0]], on_update=list(si.on_update)
            )
            for w in waits[1:]:
                nop = nc.sync.nop(nofuse=True)
                nop.ins.sync_info = mybir.SyncInfo(on_wait=[w], on_update=[])

        nc.all_engine_barrier()
        assert self.sems is not None
        popped = nc._tile_sem_poison_stack.pop()
        assert popped is self._sem_poison
        # Distributed clear_and_free: the stock path emits one ~115ns
        # sem-clear per semaphore on a single queue (~6us serial tail for
        # ~50 sems).  Spread the clears round-robin over all five engines
        # (we are between two all-engine barriers, so this is race-free).
        sems = list(self.sems.allocated().values())
        if sems:
            sem_nums = [s.num if hasattr(s, "num") else s for s in sems]
            for r in bass.compact_to_ranges(sem_nums):
                assert nc._state.free_isdisjoint(r)
                nc.gpsimd.dma_reset(r)
            engines = [nc.gpsimd, nc.tensor, nc.vector, nc.scalar, nc.sync]
            for i, s in enumerate(sorted(sem_nums)):
                engines[i % len(engines)].sem_clear(range(s, s + 1))
            nc._state.prepend_free_semaphores(sem_nums)
            for poison_set in nc._tile_sem_poison_stack:
                poison_set.update(sem_nums)
        nc.all_engine_barrier()


def _split_excess_waits(nc, max_waits=1):
    """walrus (this build) rejects instructions with more than a couple of
    semaphore waits.  Move excess waits onto single-wait NOPs inserted just
    before the offending instruction on the same engine."""
    for fn in nc.m.functions:
        for bb in fn.blocks:
            new_insts = []
            for inst in bb.instructions:
                si = inst.sync_info
                waits = list(si.on_wait) if si is not None else []
                if len(waits) > max_waits:
                    head, keep = waits[:-max_waits], waits[-max_waits:]
                    for i, w in enumerate(head):
                        nop = mybir.InstNoOp(
                            name=f"{inst.name}-sw{i}",
                            engine=inst.engine,
                            bass_nofuse=True,
                            sync_info=mybir.SyncInfo(on_wait=[w], on_update=[]),
                        )
                        nc.register_instruction(nop, overwrite=True)
                        new_insts.append(nop)
                    inst.sync_info = mybir.SyncInfo(
                        on_wait=keep, on_update=list(si.on_update)
                    )
                new_insts.append(inst)
            bb.instructions[:] = new_insts


def build_nc():
    """Build the per-core Bass graph (SPMD: same graph on all cores)."""
    nc = bass.Bass("TRN2", target_bir_lowering=False, debug=False)
    dec = nc.declare_dram_parameter("decoder_output", (B_SH, H), F32, isOutput=False)
    etd = nc.declare_dram_parameter("enc_et", (B_SH, 128, NSUB, H), FP8, isOutput=False)
    encTd = nc.declare_dram_parameter("enc_tp", (B_SH, 128, HC, S), FP8, isOutput=False)
    WeTd = nc.declare_dram_parameter("WeT8", (128, HC, A), FP8, isOutput=False)
    WdTd = nc.declare_dram_parameter("WdT", (128, HC, A), BF16, isOutput=False)
    decTd = nc.declare_dram_parameter("decT", (128, HC, B_SH), BF16, isOutput=False)
    Vcd = nc.declare_dram_parameter("Vc8", (128, AC, 16), FP8, isOutput=False)
    bdd = nc.declare_dram_parameter("b_dec", (A,), F32, isOutput=False)
    bed = nc.declare_dram_parameter("b_enc", (A,), F32, isOutput=False)
    bvd = nc.declare_dram_parameter("bv_col", (128, 1), F32, isOutput=False)
    out = nc.declare_dram_parameter("out", (B_SH, 2 * H), F32, isOutput=True)

    with ExitStack() as ctx:
        tc = ctx.enter_context(_SplitDrainTileContext(nc))
        consts = ctx.enter_context(tc.tile_pool(name="consts", bufs=1))

        WeT8 = consts.tile([128, HC, A], FP8)
        Vc8 = consts.tile([128, AC, 16], FP8)
        WdT = consts.tile([128, HC, A], BF16)
        decT = consts.tile([128, HC, B_SH], BF16)
        dterm = consts.tile([128, AC, B_SH], F32)   # dec_p + b_dec + b_enc
        bv_raw = consts.tile([128, 1], F32)
        bvt = consts.tile([128, 1], F32)            # b_v - SHIFT, per partition
        bd_row = consts.tile([1, A], F32)
        be_row = consts.tile([1, A], F32)
        ecol8 = consts.tile([128, NP, 2 * SUB, 16], FP8)  # e columns, stride-16
        lacc = consts.tile([128, B_SH], F32)        # per-partition l partials
        ones_col = consts.tile([128, 1], F32)
        ones_row = consts.tile([1, B_SH], F32)
        junk = consts.tile([128, 128], BF16)
        zz = consts.tile([1, 2], F32)
        encT_sb = consts.tile([128, B_SH, HC, S], FP8)
        et_sb = consts.tile([128, B_SH, NSUB, H], FP8)

        # ---- small loads lead the sync ring so they land before the bulk
        # encoder streams start hogging the DMA engines (dterm and the first
        # projection gate on them) ----
        nc.sync.dma_start(out=WeT8, in_=WeTd[:, :, :])
        nc.sync.dma_start(out=WdT, in_=WdTd[:, :, :])
        nc.sync.dma_start(out=decT, in_=decTd[:, :, :])
        nc.sync.dma_start(out=bd_row, in_=bdd[None, :])
        nc.sync.dma_start(out=be_row, in_=bed[None, :])

        # ---- memsets / ACT table preload ----
        nc.gpsimd.memset(junk, 0.5)
        nc.gpsimd.memset(ones_col, 1.0)
        nc.gpsimd.memset(ones_row, 1.0)
        nc.gpsimd.memset(zz, 0.0)
        nc.scalar.activation(out=zz[:, 1:2], in_=zz[:, 0:1], func=AF.Tanh,
                             bias=zz[:, 0:1])

        # ---- big encoder streams on the sync ring (8KB/partition each).
        # encT rows lead et rows by ~1.5 rows: the projection consumes
        # encT[b] a full row-period before ctx needs et[b]. ----
        # row 0's first half (pair 0) lands ~1.5us before the full row
        # would, so the first projection starts that much earlier; the two
        # non-critical small loads ride behind it.
        nc.sync.dma_start(out=encT_sb[:, 0, :, 0:S//2], in_=encTd[0][:, :, 0:S//2])
        nc.sync.dma_start(out=encT_sb[:, 0, :, S//2:S], in_=encTd[0][:, :, S//2:S])
        nc.sync.dma_start(out=Vc8, in_=Vcd[:, :, :])
        nc.sync.dma_start(out=bv_raw, in_=bvd[:, :])
        nc.sync.dma_start(out=encT_sb[:, 1], in_=encTd[1])
        for b in range(B_SH):
            nc.sync.dma_start(out=et_sb[:, b], in_=etd[b])
            if b + 2 < B_SH:
                nc.sync.dma_start(out=encT_sb[:, b + 2], in_=encTd[b + 2])

        # decoder passthrough (DRAM -> DRAM)
        nc.gpsimd.dma_start(out=out[:, H : 2 * H], in_=dec[:, :])

        # ---- PE warm-up + dterm (scoped PSUM) ----
        with tc.tile_pool(name="wps", bufs=2, space="PSUM") as wps:
            warm = wps.tile([128, 128], F32, tag="warm")
            for _ in range(N_WARM):
                nc.tensor.matmul(warm, junk, junk, start=True, stop=True)
            for ac in range(AC):
                dt_ps = wps.tile([128, B_SH], F32, tag="dt")
                for hc in range(HC):
                    nc.tensor.matmul(
                        dt_ps, WdT[:, hc, ts(ac, 128)], decT[:, hc, :],
                        start=(hc == 0), stop=False,
                    )
                nc.tensor.matmul(dt_ps, bd_row[0:1, ts(ac, 128)], ones_row,
                                 start=False, stop=False)
                nc.tensor.matmul(dt_ps, be_row[0:1, ts(ac, 128)], ones_row,
                                 start=False, stop=True)
                nc.vector.tensor_copy(dterm[:, ac, :], dt_ps)
        nc.vector.tensor_scalar_add(bvt, bv_raw, -SHIFT)

        # ---- main pools ----
        hT_pool = ctx.enter_context(tc.tile_pool(name="hT", bufs=2))
        sm_pool = ctx.enter_context(tc.tile_pool(name="small", bufs=2))
        pp_pool = ctx.enter_context(tc.tile_pool(name="pp", bufs=2, space="PSUM"))
        scT_pool = ctx.enter_context(tc.tile_pool(name="scT", bufs=2, space="PSUM"))
        lps_pool = ctx.enter_context(tc.tile_pool(name="lps", bufs=1, space="PSUM"))
        ctx_pool = ctx.enter_context(tc.tile_pool(name="ctxp", bufs=1, space="PSUM"))

        WeT8r = WeT8.rearrange("p hc (ac f) -> p hc ac f", f=128)

        hTs = {}       # pair g -> hT tile
        scTs = {}      # row b -> scoresT psum tile [128, 16]
        ctxs = {}      # row b -> ctx psum tile [1, H]

        def emit_scoresT(g):
            """V-dot of pair g as column matmuls: stationary = hT slices
            (a-contraction), moving = V column, out [s%128, block]."""
            b, p = divmod(g, NP)
            if p == 0:
                scT = scT_pool.tile([128, 2 * 2 * SUB], F32, tag="scT")
                scTs[b] = scT
            scT = scTs[b]
            hT = hTs.pop(g)
            for sb in range(2 * SUB):
                t, csb = divmod(sb, SUB)
                col = 2 * SUB * p + sb
                for acp in (0, 2):
                    nc.tensor.matmul(
                        scT[:, col : col + 1],
                        hT[:, acp : acp + 2, t, ts(csb, 128)],
                        Vc8[:, acp : acp + 2, 0:1],
                        start=(acp == 0), stop=(acp == 2),
                        perf_mode=DR, skip_group_check=True,
                    )

        def make_scoresT_units(g):
            """The 16 V-dot units of pair g as thunks, ordered acp=0 units
            first then acp=2 (per-column start before stop).  Interleaved one
            per projection matmul of the NEXT pair, each unit's ~53ns
            hT-stationary LDWEIGHTS hides under the in-flight 216ns
            projection matmul instead of serializing (16x78ns -> ~16x25ns
            of PE time per pair)."""
            b, p = divmod(g, NP)
            if p == 0:
                scT = scT_pool.tile([128, 2 * 2 * SUB], F32, tag="scT")
                scTs[b] = scT
            scT = scTs[b]
            hT = hTs.pop(g)
            units = []
            for acp in (0, 2):
                for sb in range(2 * SUB):
                    t, csb = divmod(sb, SUB)
                    col = 2 * SUB * p + sb

                    def unit(t=t, csb=csb, col=col, acp=acp, scT=scT, hT=hT):
                        nc.tensor.matmul(
                            scT[:, col : col + 1],
                            hT[:, acp : acp + 2, t, ts(csb, 128)],
                            Vc8[:, acp : acp + 2, 0:1],
                            start=(acp == 0), stop=(acp == 2),
                            perf_mode=DR, skip_group_check=True,
                        )

                    units.append(unit)
            return units

        def emit_exp(b):
            """One exp for the whole row: [128, 16] -> e columns (fp8) +
            per-partition l partials via the activation accumulator."""
            nc.scalar.activation(
                out=ecol8[:, :, :, 0],
                in_=scTs[b],
                func=AF.Exp,
                bias=bvt,
                scale=1.0 / W_SCALE,
                accum_out=lacc[:, b : b + 1],
            )

        def emit_ctx_and_out(b):
            """Context reduction for row b (8 DR matmuls) + normalization."""
            ctx_ps = ctx_pool.tile([1, H], F32, tag="ctx")
            for p in range(NP):
                for q in range(SUB):
                    nc.tensor.matmul(
                        ctx_ps,
                        ecol8[:, p, 2 * q : 2 * q + 2, 0:1],
                        et_sb[:, b, 8 * p + 2 * q : 8 * p + 2 * q + 2, :],
                        start=(p == 0 and q == 0),
                        stop=(p == NP - 1 and q == SUB - 1),
                        perf_mode=DR, skip_group_check=True,
                    )
            l_ps = lps_pool.tile([1, 1], F32, tag="lps")
            nc.tensor.matmul(l_ps, lacc[:, b : b + 1], ones_col,
                             start=True, stop=True, skip_group_check=True)
            linv = sm_pool.tile([1, 1], F32, tag="linv")
            nc.vector.reciprocal(linv, l_ps)
            orow = sm_pool.tile([1, H], F32, tag="orow")
            nc.vector.tensor_scalar_mul(orow, ctx_ps, linv)
            nc.gpsimd.dma_start(out=out[b : b + 1, 0:H], in_=orow)

        pend_units = []
        for g in range(TOTAL):
            b, p = divmod(g, NP)
            # During the pipeline-fill bodies (g<4) the ACT engine still lags
            # the PE, so interleaved units would stall mid-projection; emit
            # them as a block at the end of the body instead.
            interleave = g >= 4
            hT = hT_pool.tile([128, AC, 2, ST], FP8, tag="hT")
            hTs[g] = hT
            for ac in range(AC):
                pp = pp_pool.tile([128, 2, ST], F32, tag="pp")
                for t in range(2):
                    for hq in (0, 2):
                        nc.tensor.matmul(
                            pp[:, t, :],
                            WeT8r[:, hq : hq + 2, ac, :],
                            encT_sb[:, b, hq : hq + 2, ts(2 * p + t, ST)],
                            start=(hq == 0), stop=(hq == 2),
                            perf_mode=DR, skip_group_check=True,
                        )
                        if interleave and pend_units:
                            pend_units.pop(0)()
                nc.scalar.activation(
                    out=hT[:, ac],
                    in_=pp,
                    func=AF.Tanh,
                    bias=dterm[:, ac, b : b + 1],
                    scale=1.0 / W_SCALE,
                )
                # exp(b-1) goes after tanh(g,3) on the ACT queue: the last
                # interleaved V-dot unit of row b-1 lands just before the
                # PE finishes this body's projection.  ctx(b-1) waits one
                # more body (p==1, ac0) so the exp is long done when the
                # PE reaches it.
                if ac == 3 and p == 0 and b > 0:
                    emit_exp(b - 1)
                if ac == 0 and p == 1 and b > 0:
                    emit_ctx_and_out(b - 1)
            while pend_units:
                pend_units.pop(0)()
            pend_units = make_scoresT_units(g)

        while pend_units:
            pend_units.pop(0)()
        emit_exp(B_SH - 1)
        emit_ctx_and_out(B_SH - 1)

    _split_excess_waits(nc)
    return nc


_CACHED = {}


def _get_nc():
    if "nc" not in _CACHED:
        _CACHED["nc"] = build_nc()
    return _CACHED["nc"]


def _prep(inputs):
    """Host-side staging: fp8 partition-major copies of the big operands in
    the layouts the device matmuls consume directly."""
    import ml_dtypes

    f8 = ml_dtypes.float8_e4m3
    bf = ml_dtypes.bfloat16

    dec = np.ascontiguousarray(np.asarray(inputs["decoder_output"], np.float32))
    enc = np.asarray(inputs["encoder_output"], np.float32)
    Wd = np.asarray(inputs["W_dec"], np.float32)
    We = np.asarray(inputs["W_enc"], np.float32)
    bd = np.ascontiguousarray(np.asarray(inputs["b_dec"], np.float32))
    be = np.ascontiguousarray(np.asarray(inputs["b_enc"], np.float32))
    V = np.asarray(inputs["V"], np.float32)
    bv = np.asarray(inputs["b_v"], np.float32)

    enc8 = enc.astype(f8)
    # [b, s%128... ] -> partition-major: et[b, p, sub, h] = enc[b, sub*128+p, h]
    et = np.ascontiguousarray(enc8.reshape(B, NSUB, 128, H).transpose(0, 2, 1, 3))
    # encT[b, p, hc, s] = enc[b, s, hc*128+p]
    encT = np.ascontiguousarray(
        enc8.transpose(0, 2, 1).reshape(B, HC, 128, S).transpose(0, 2, 1, 3)
    )
    WeT8 = np.ascontiguousarray(
        (We.T * W_SCALE).reshape(HC, 128, A).transpose(1, 0, 2)
    ).astype(f8)
    WdT = np.ascontiguousarray(
        Wd.T.reshape(HC, 128, A).transpose(1, 0, 2)
    ).astype(bf)
    Vc8 = np.zeros((128, AC, 16), f8)
    Vc8[:, :, 0] = ((V[0] * W_SCALE).reshape(AC, 128).T).astype(f8)
    bv_col = np.ascontiguousarray(
        np.broadcast_to(bv.reshape(1, 1), (128, 1)).astype(np.float32)
    )
    return {
        "dec": dec, "et": et, "encT": encT, "WeT8": WeT8, "WdT": WdT,
        "Vc8": Vc8, "bd": bd, "be": be, "bv_col": bv_col,
    }


def make_in_maps(ins):
    import ml_dtypes

    bf = ml_dtypes.bfloat16
    in_maps = []
    for c in range(N_CORES):
        sl = slice(c * B_SH, (c + 1) * B_SH)
        dec_c = ins["dec"][sl]
        decT_c = np.ascontiguousarray(
            dec_c.T.reshape(HC, 128, B_SH).transpose(1, 0, 2)
        ).astype(bf)
        in_maps.append(
            {
                "decoder_output": dec_c,
                "enc_et": ins["et"][sl],
                "enc_tp": ins["encT"][sl],
                "WeT8": ins["WeT8"],
                "WdT": ins["WdT"],
                "decT": decT_c,
                "Vc8": ins["Vc8"],
                "b_dec": ins["bd"],
                "b_enc": ins["be"],
                "bv_col": ins["bv_col"],
            }
        )
    return in_maps


def kernel(**inputs) -> np.ndarray:
    ins = _prep(inputs)
    nc = _get_nc()
    in_maps = make_in_maps(ins)
    # The device occasionally comes up wedged from a previous process
    # (NRT_EXEC_UNIT_UNRECOVERABLE); a failed attempt clears it, so retry.
    last_err = None
    for _attempt in range(3):
        try:
            res = run_bass_kernel_spmd(nc, in_maps, core_ids=list(range(N_CORES)))
            return np.concatenate(
                [res.results[c]["out"] for c in range(N_CORES)], axis=0
            )
        except Exception as e:  # noqa: BLE001
            last_err = e
            import time

            time.sleep(5)
    raise last_err
